# revision 19
# baseline (speedup 1.0000x reference)
"""NonLocal2D block (SAGAN-style non-local attention) on 8 Trainium2 cores.

Data-parallel over batch: core b computes batch element b entirely on-chip.

Math (per batch, N = 64*64 = 4096):
  f = Wf@x+bf [16,N], g = Wg@x+bg [16,N], h = Wh@x+bh [128,N]
  S = f^T g [N,N]; A = softmax_rows(S); att = h @ A; out = x + gamma*att

Decomposition (per core), using att[c,m] = sum_n hT'[n,c] * E[n,m] with
E = exp(S) and hT'[n,c] = h[c,n] * gamma/D[n], D[n] = sum_m E[n,m]:

  32 row-strips of 128 n's, in windows of 4. Per strip:
    S_strip = f_strip^T @ g          PE, K=16 bf16 matmuls -> PSUM
    E_strip = exp(S_strip)           ACT (the roofline: 16.7M exps/core),
                                     PSUM->SBUF bf16; last 2 chunks also
                                     emit accum_out partial row-sums
    D rowsum                         DVE reduce (first 2048 cols) + ACT accums
    hT = x_strip^T @ WhT + 1 (x) bh  PE (K=128 + K=1 rank-1 bias)
    hT' = hT * (gamma/D)             DVE, PSUM->SBUF bf16
  Attended accumulates over 4-strip groups in PSUM (K-chained matmuls) and
  is folded into an SBUF accumulator by DVE; group g's attended matmuls run
  during window g+1 so ACT never starves. The residual x is fused into the
  first fold; tail stores per 512-column block.

All tensors that are produced/consumed blockwise are split into per-block
tiles because Tile tracks dependencies per tile, not per slice.
"""

import numpy as np
import ml_dtypes

import concourse.bass as bass
import concourse.bacc as bacc
import concourse.tile as tile
import concourse.mybir as mybir
from concourse.bass_utils import run_bass_kernel_spmd

B, C, W, H = 8, 128, 64, 64
N = W * H          # 4096
CP = 16            # f/g channels
P = 128
NSTRIP = N // P    # 32
GROUP = 4          # strips per window / attended K-chain
NWIN = NSTRIP // GROUP      # 8
MBLK = 512
NMB = N // MBLK    # 8
# exp call chunks (psS tiles are [128,1536] = 3 banks x 2 bufs):
CHUNKS = [(0, 1536), (1536, 1536), (3072, 1024)]
# rowsum: chunks 0..1 (3072 cols) via DVE add-tree, chunk 2 via ACT accum

F32 = mybir.dt.float32
BF16 = mybir.dt.bfloat16
F16 = mybir.dt.float16
EXP = mybir.ActivationFunctionType.Exp
AX = mybir.AxisListType.X
MUL = mybir.AluOpType.mult

_NC = None


def _build():
    nc = bacc.Bacc(None, target_bir_lowering=False)
    x32 = nc.dram_tensor("x32", [P, N], F32, kind="ExternalInput")
    xbf = nc.dram_tensor("xbf", [P, N], BF16, kind="ExternalInput")
    # wpack: [wf^T rep | wg^T rep | wh^T | row0: bh] packed on host
    wpack = nc.dram_tensor("wpack", [P, 4 * P], BF16, kind="ExternalInput")
    # fpack: [bf4 | bg4 | gamma (pre-broadcast)] packed on host
    fpack = nc.dram_tensor("fpack", [P, 3], F32, kind="ExternalInput")
    out = nc.dram_tensor("out", [P, N], F32, kind="ExternalOutput")

    with tile.TileContext(nc) as tc:
        with (
            tc.tile_pool(name="consts", bufs=1) as consts,
            tc.tile_pool(name="epool", bufs=2 * GROUP + 2) as epool,
            tc.tile_pool(name="hpool", bufs=2 * GROUP + 2) as hpool,
            tc.tile_pool(name="small", bufs=6) as small,
            tc.tile_pool(name="psS", bufs=2, space="PSUM") as psS,
            tc.tile_pool(name="psA", bufs=2, space="PSUM") as psA,
        ):
            # ---- interleave input DMAs across the two DGE paths so xbf
            # block 0 and the packed weights land first.
            wpack_s = consts.tile([P, 4 * P], BF16)
            fpack_s = consts.tile([P, 3], F32)
            xbf_t = [consts.tile([P, MBLK], BF16, tag=f"xbf{j}", name=f"xbf{j}")
                     for j in range(NMB)]
            nc.gpsimd.dma_start(xbf_t[0][:], xbf[:, 0:MBLK])
            nc.sync.dma_start(fpack_s[:], fpack[:])
            nc.gpsimd.dma_start(wpack_s[:], wpack[:])
            nc.sync.dma_start(xbf_t[1][:], xbf[:, MBLK:2 * MBLK])
            for j in range(2, NMB):
                eng = nc.gpsimd if j % 2 == 0 else nc.sync
                eng.dma_start(xbf_t[j][:], xbf[:, j * MBLK:(j + 1) * MBLK])

            wft4_s = wpack_s[:, 0:P]
            wgt4_s = wpack_s[:, P:2 * P]
            wht_s = wpack_s[:, 2 * P:3 * P]
            bhr_s = wpack_s[0:1, 3 * P:4 * P]
            bf4_s = fpack_s[:, 0:1]
            bg4_s = fpack_s[:, 1:2]
            gam_s = fpack_s[:, 2:3]
            ones_s = consts.tile([1, P], BF16)
            nc.vector.memset(ones_s[:], 1.0)

            f4_t = [consts.tile([P, MBLK], BF16, tag=f"f4{j}", name=f"f4{j}")
                    for j in range(NMB)]
            g4_t = [consts.tile([P, wd], BF16, tag=f"g4{c}", name=f"g4{c}")
                    for c, (off, wd) in enumerate(CHUNKS)]
            att_t = [consts.tile([P, MBLK], F32, tag=f"att{j}", name=f"att{j}")
                     for j in range(NMB)]

            # ---- f/g 1x1 convs; bias added on the PSUM->SBUF copy.
            # Order matters: strip 0 needs f block 0 and the g chunks in
            # order, so emit those first; remaining f blocks trail.
            IDENT = mybir.ActivationFunctionType.Identity

            def fg_block(j, which, via_act=False):
                ps = psA.tile([P, MBLK], F32, tag="att")
                if which == "f":
                    dst, b = f4_t[j][:], bf4_s
                    nc.tensor.matmul(ps[:], wft4_s, xbf_t[j][:],
                                     start=True, stop=True)
                else:
                    c = next(i for i, (off, wd) in enumerate(CHUNKS)
                             if off <= j * MBLK < off + wd)
                    o = j * MBLK - CHUNKS[c][0]
                    dst = g4_t[c][:, o:o + MBLK]
                    b = bg4_s
                    nc.tensor.matmul(ps[:], wgt4_s, xbf_t[j][:],
                                     start=True, stop=True)
                if via_act:
                    # ACT is idle during startup; Identity shares Exp's table
                    nc.scalar.activation(out=dst, in_=ps[:], func=IDENT,
                                         bias=b, scale=1.0)
                else:
                    nc.vector.tensor_scalar_add(out=dst, in0=ps[:], scalar1=b)

            # Only what strip 0 chunk 0 needs; the rest is emitted
            # just-in-time inside the strip loop (PE executes in order, so
            # early emission would delay strip 0's S matmuls).
            fg_block(0, "f", via_act=True)
            fg_block(0, "g", via_act=True)
            fg_block(1, "g", via_act=True)
            fg_block(2, "g", via_act=True)

            # x32 only needed for the first folds; per-block tiles
            x32_t = []
            for j in range(NMB):
                t = consts.tile([P, MBLK], F32, tag=f"x32{j}", name=f"x32{j}")
                eng = nc.gpsimd if j % 2 == 0 else nc.sync
                eng.dma_start(t[:], x32[:, j * MBLK:(j + 1) * MBLK])
                x32_t.append(t)

            def att_block(j, group, first):
                """att[j] (+)= sum_k hT'_k^T @ E_k[:, blk j]; first fold also
                adds the residual x."""
                blk = slice(j * MBLK, (j + 1) * MBLK)
                pa = psA.tile([P, MBLK], F32, tag="att")
                for k, (hk, ek) in enumerate(group):
                    nc.tensor.matmul(pa[:], hk[:], ek[:, blk],
                                     start=(k == 0), stop=(k == len(group) - 1))
                if first:
                    nc.vector.tensor_add(out=att_t[j][:], in0=pa[:],
                                         in1=x32_t[j][:])
                else:
                    nc.vector.tensor_add(out=att_t[j][:], in0=att_t[j][:],
                                         in1=pa[:])

            groups = [[] for _ in range(NWIN)]
            for w in range(NWIN):
                for i in range(GROUP):
                    s = w * GROUP + i
                    if i == 1 and w < NWIN - 1:
                        fg_block(w + 1, "f")
                    # hT = x_strip^T @ WhT + ones (x) bh  -> [n, c]; copied
                    # to SBUF right away so the psA slot frees quickly
                    ph = psA.tile([P, MBLK], F32, tag="att", name="ph")
                    nc.tensor.matmul(ph[:, 0:P], xbf_t[s // 4][:, (s % 4) * P:
                                                               (s % 4 + 1) * P],
                                     wht_s, start=True, stop=False)
                    nc.tensor.matmul(ph[:, 0:P], ones_s[:], bhr_s,
                                     start=False, stop=True)
                    ht0 = hpool.tile([P, P], BF16, tag="ht0", name="ht0")
                    nc.vector.tensor_copy(out=ht0[:], in_=ph[:, 0:P])
                    # S strip (K=16) -> exp -> E strip (+ accum partial sums)
                    e = epool.tile([P, N], BF16, tag="E")
                    # last strip: all chunks via ACT accum so the tail's
                    # attended chains aren't gated on the DVE rowsum tree
                    dvc = 0 if s == NSTRIP - 1 else len(CHUNKS) - 1
                    accs = small.tile([P, len(CHUNKS)], F32, tag="accs")
                    fl = f4_t[s // 4][:, (s % 4) * P:(s % 4 + 1) * P]
                    fsl = fl[0:CP, :]
                    for cix, (coff, cwd) in enumerate(CHUNKS):
                        if s == 0 and cix >= 1:
                            for gb in range(coff // MBLK,
                                            (coff + cwd) // MBLK):
                                fg_block(gb, "g")
                        sps = psS.tile([P, 1536], F32)
                        for half in range(cwd // MBLK):
                            off = half * MBLK
                            nc.tensor.matmul(
                                sps[:, off:off + MBLK],
                                fsl,
                                g4_t[cix][0:CP, off:off + MBLK],
                                start=True, stop=True)
                        eout = e[:, coff:coff + cwd]
                        if cix < dvc:
                            nc.scalar.activation(out=eout, in_=sps[:, 0:cwd],
                                                 func=EXP)
                        else:
                            nc.scalar.activation(
                                out=eout, in_=sps[:, 0:cwd], func=EXP,
                                accum_out=accs[:, cix - dvc:cix - dvc + 1])
                    # D rowsum over chunks 0..2 via 2-byte DVE add-tree (2x
                    # packed mode: ~2.2us vs 3.3us plain reduce), chunk 3 from
                    # the ACT accumulator.  3072 -> 1536 -> 768 -> reduce.
                    d = small.tile([P, 1], F32, tag="d")
                    if dvc:
                        t1 = small.tile([P, 1536], F16, tag="t1")
                        nc.vector.tensor_add(out=t1[:], in0=e[:, 0:1536],
                                             in1=e[:, 1536:3072])
                        t2 = small.tile([P, 768], F16, tag="t2")
                        nc.vector.tensor_add(out=t2[:], in0=t1[:, 0:768],
                                             in1=t1[:, 768:1536])
                        dd = small.tile([P, 1], F32, tag="dd")
                        nc.vector.reduce_sum(out=dd[:], in_=t2[:], axis=AX)
                        nc.vector.tensor_add(out=d[:], in0=dd[:],
                                             in1=accs[:, 0:1])
                    else:
                        nc.vector.tensor_add(out=d[:], in0=accs[:, 0:1],
                                             in1=accs[:, 1:2])
                        nc.vector.tensor_add(out=d[:], in0=d[:],
                                             in1=accs[:, 2:3])
                    rd = small.tile([P, 1], F32, tag="rd")
                    nc.vector.reciprocal(out=rd[:], in_=d[:])
                    hts = hpool.tile([P, P], BF16, tag="hts")
                    nc.vector.tensor_scalar(out=hts[:], in0=ht0[:],
                                            scalar1=rd[:], scalar2=gam_s,
                                            op0=MUL, op1=MUL)
                    groups[w].append((hts, e))
                    # attended for the previous window's group (keeps PE busy
                    # while ACT chews the current window's exps)
                    if w >= 1:
                        for j in (2 * i, 2 * i + 1):
                            att_block(j, groups[w - 1], first=(w == 1))

            # tail: attended for the last group, then store. pa tiles come
            # from both psum pools (psS is idle now) for deeper overlap.
            for j in range(NMB):
                pool = psS if j % 2 == 0 else psA
                pa = pool.tile([P, 1536 if pool is psS else MBLK], F32,
                               tag="sps" if pool is psS else "att",
                               name=f"tailpa{j}")
                g = groups[-1]
                for k, (hk, ek) in enumerate(g):
                    nc.tensor.matmul(pa[:, 0:MBLK], hk[:],
                                     ek[:, j * MBLK:(j + 1) * MBLK],
                                     start=(k == 0), stop=(k == len(g) - 1))
                nc.vector.tensor_add(out=att_t[j][:], in0=att_t[j][:],
                                     in1=pa[:, 0:MBLK])
                nc.sync.dma_start(out[:, j * MBLK:(j + 1) * MBLK], att_t[j][:])

    nc.compile()
    return nc


def _get_nc():
    global _NC
    if _NC is None:
        _NC = _build()
    return _NC


def _prep_weights(Wf, bf, Wg, bg, Wh, bh, gamma):
    bf16 = ml_dtypes.bfloat16
    wft4 = np.zeros((P, P), np.float32)
    wgt4 = np.zeros((P, P), np.float32)
    bf4 = np.zeros((P, 1), np.float32)
    bg4 = np.zeros((P, 1), np.float32)
    for i in range(4):
        wft4[:, 32 * i:32 * i + CP] = Wf.T
        wgt4[:, 32 * i:32 * i + CP] = Wg.T
        bf4[32 * i:32 * i + CP, 0] = bf
        bg4[32 * i:32 * i + CP, 0] = bg
    wpack = np.zeros((P, 4 * P), np.float32)
    wpack[:, 0:P] = wft4
    wpack[:, P:2 * P] = wgt4
    wpack[:, 2 * P:3 * P] = Wh.T
    wpack[0, 3 * P:4 * P] = bh
    fpack = np.zeros((P, 3), np.float32)
    fpack[:, 0:1] = bf4
    fpack[:, 1:2] = bg4
    fpack[:, 2] = np.float32(np.asarray(gamma).reshape(()))
    return {"wpack": wpack.astype(bf16), "fpack": fpack}


def make_in_maps(x, Wf, bf, Wg, bg, Wh, bh, gamma):
    bf16 = ml_dtypes.bfloat16
    wmap = _prep_weights(np.asarray(Wf), np.asarray(bf), np.asarray(Wg),
                         np.asarray(bg), np.asarray(Wh), np.asarray(bh),
                         np.asarray(gamma))
    xf = np.ascontiguousarray(np.asarray(x, np.float32).reshape(B, C, N))
    in_maps = []
    for b in range(B):
        m = dict(wmap)
        m["x32"] = xf[b]
        m["xbf"] = xf[b].astype(bf16)
        in_maps.append(m)
    return in_maps


def kernel(x, Wf, bf, Wg, bg, Wh, bh, gamma):
    nc = _get_nc()
    in_maps = make_in_maps(x, Wf, bf, Wg, bg, Wh, bh, gamma)
    res = run_bass_kernel_spmd(nc, in_maps, core_ids=list(range(B)))
    out = np.stack([res.results[b]["out"] for b in range(B)], axis=0)
    return out.reshape(B, C, W, H).astype(np.float32)


# revision 26
# speedup vs baseline: 8280.2327x; 8280.2327x over previous
"""NonLocal2D block (SAGAN-style non-local attention) on 8 Trainium2 cores.

Data-parallel over batch: core b computes batch element b entirely on-chip.

Math (per batch, N = 64*64 = 4096):
  f = Wf@x+bf [16,N], g = Wg@x+bg [16,N], h = Wh@x+bh [128,N]
  S = f^T g [N,N]; A = softmax_rows(S); att = h @ A; out = x + gamma*att

Decomposition (per core), using att[c,m] = sum_n hT'[n,c] * E[n,m] with
E = exp(S) and hT'[n,c] = h[c,n] * gamma/D[n], D[n] = sum_m E[n,m]:

  32 row-strips of 128 n's, in windows of 4. Per strip:
    S_strip = f_strip^T @ g          PE, K=16 bf16 matmuls -> PSUM
    E_strip = exp(S_strip)           ACT (the roofline: 16.7M exps/core),
                                     PSUM->SBUF bf16; last 2 chunks also
                                     emit accum_out partial row-sums
    D rowsum                         DVE reduce (first 2048 cols) + ACT accums
    hT = x_strip^T @ WhT + 1 (x) bh  PE (K=128 + K=1 rank-1 bias)
    hT' = hT * (gamma/D)             DVE, PSUM->SBUF bf16
  Attended accumulates over 4-strip groups in PSUM (K-chained matmuls) and
  is folded into an SBUF accumulator by DVE; group g's attended matmuls run
  during window g+1 so ACT never starves. The residual x is fused into the
  first fold; tail stores per 512-column block.

All tensors that are produced/consumed blockwise are split into per-block
tiles because Tile tracks dependencies per tile, not per slice.
"""

import numpy as np
import ml_dtypes

import concourse.bass as bass
import concourse.bacc as bacc
import concourse.tile as tile
import concourse.mybir as mybir
from concourse.bass_utils import run_bass_kernel_spmd

B, C, W, H = 8, 128, 64, 64
N = W * H          # 4096
CP = 16            # f/g channels
P = 128
NSTRIP = N // P    # 32
GROUP = 4          # strips per window / attended K-chain
NWIN = NSTRIP // GROUP      # 8
MBLK = 512
NMB = N // MBLK    # 8
# exp call chunks (psS tiles are [128,1536] = 3 banks x 2 bufs):
CHUNKS = [(0, 1536), (1536, 1536), (3072, 1024)]
# rowsum: chunks 0..1 (3072 cols) via DVE add-tree, chunk 2 via ACT accum

F32 = mybir.dt.float32
BF16 = mybir.dt.bfloat16
F16 = mybir.dt.float16
EXP = mybir.ActivationFunctionType.Exp
AX = mybir.AxisListType.X
MUL = mybir.AluOpType.mult

_NC = None


def _build():
    nc = bacc.Bacc(None, target_bir_lowering=False)
    x32 = nc.dram_tensor("x32", [P, N], F32, kind="ExternalInput")
    xbf = nc.dram_tensor("xbf", [P, N], BF16, kind="ExternalInput")
    # wpack: [wf^T rep | wg^T rep | wh^T | row0: bh] packed on host
    wpack = nc.dram_tensor("wpack", [P, 4 * P], BF16, kind="ExternalInput")
    # fpack: [bf4 | bg4 | gamma (pre-broadcast)] packed on host
    fpack = nc.dram_tensor("fpack", [P, 3], F32, kind="ExternalInput")
    out = nc.dram_tensor("out", [P, N], F32, kind="ExternalOutput")

    with tile.TileContext(nc) as tc:
        with (
            tc.tile_pool(name="consts", bufs=1) as consts,
            tc.tile_pool(name="epool", bufs=2 * GROUP + 4) as epool,
            tc.tile_pool(name="hpool", bufs=2 * GROUP + 6) as hpool,
            tc.tile_pool(name="small", bufs=8) as small,
            tc.tile_pool(name="psS", bufs=2, space="PSUM") as psS,
            tc.tile_pool(name="psA", bufs=2, space="PSUM") as psA,
        ):
            # ---- interleave input DMAs across the two DGE paths so xbf
            # block 0 and the packed weights land first.
            wpack_s = consts.tile([P, 4 * P], BF16)
            fpack_s = consts.tile([P, 3], F32)
            xbf_t = [consts.tile([P, MBLK], BF16, tag=f"xbf{j}", name=f"xbf{j}")
                     for j in range(NMB)]
            nc.sync.dma_start(wpack_s[:], wpack[:])
            nc.gpsimd.dma_start(xbf_t[0][:], xbf[:, 0:MBLK])
            nc.sync.dma_start(fpack_s[:], fpack[:])
            nc.sync.dma_start(xbf_t[1][:], xbf[:, MBLK:2 * MBLK])
            for j in range(2, NMB):
                eng = nc.gpsimd if j % 2 == 0 else nc.sync
                eng.dma_start(xbf_t[j][:], xbf[:, j * MBLK:(j + 1) * MBLK])

            wft4_s = wpack_s[:, 0:P]
            wgt4_s = wpack_s[:, P:2 * P]
            wht_s = wpack_s[:, 2 * P:3 * P]
            bhr_s = wpack_s[0:1, 3 * P:4 * P]
            bf4_s = fpack_s[:, 0:1]
            bg4_s = fpack_s[:, 1:2]
            gam_s = fpack_s[:, 2:3]
            ones_s = consts.tile([1, P], BF16)
            nc.vector.memset(ones_s[:], 1.0)
            neg6_s = consts.tile([P, 1], F32)
            nc.vector.memset(neg6_s[:], -6.0)
            # dummy exp with no input deps: pulls the ACT table load to t=0
            # instead of just before the first real activation
            warm = small.tile([P, 1], F32, tag="warm")
            nc.scalar.activation(out=warm[:], in_=neg6_s[:], func=EXP)

            f4_t = [consts.tile([P, MBLK], BF16, tag=f"f4{j}", name=f"f4{j}")
                    for j in range(NMB)]
            g4_t = [consts.tile([P, wd], BF16, tag=f"g4{c}", name=f"g4{c}")
                    for c, (off, wd) in enumerate(CHUNKS)]
            att_t = [consts.tile([P, MBLK], F32, tag=f"att{j}", name=f"att{j}")
                     for j in range(NMB)]

            # ---- f/g 1x1 convs; bias added on the PSUM->SBUF copy.
            # Order matters: strip 0 needs f block 0 and the g chunks in
            # order, so emit those first; remaining f blocks trail.
            IDENT = mybir.ActivationFunctionType.Identity

            def fg_block(j, which, via_act=False):
                ps = psA.tile([P, MBLK], F32, tag="att")
                if which == "f":
                    dst, b = f4_t[j][:], bf4_s
                    nc.tensor.matmul(ps[:], wft4_s, xbf_t[j][:],
                                     start=True, stop=True)
                else:
                    c = next(i for i, (off, wd) in enumerate(CHUNKS)
                             if off <= j * MBLK < off + wd)
                    o = j * MBLK - CHUNKS[c][0]
                    dst = g4_t[c][:, o:o + MBLK]
                    b = bg4_s
                    nc.tensor.matmul(ps[:], wgt4_s, xbf_t[j][:],
                                     start=True, stop=True)
                if via_act:
                    # ACT is idle during startup; Identity shares Exp's table
                    nc.scalar.activation(out=dst, in_=ps[:], func=IDENT,
                                         bias=b, scale=1.0)
                else:
                    nc.vector.tensor_scalar_add(out=dst, in0=ps[:], scalar1=b)

            # Only what strip 0 chunk 0 needs; the rest is emitted
            # just-in-time inside the strip loop (PE executes in order, so
            # early emission would delay strip 0's S matmuls).
            fg_block(0, "f", via_act=True)
            fg_block(0, "g", via_act=False)
            fg_block(1, "g", via_act=True)
            fg_block(2, "g", via_act=False)

            # x32 only needed for the first folds; per-block tiles
            x32_t = []
            for j in range(NMB):
                t = consts.tile([P, MBLK], F32, tag=f"x32{j}", name=f"x32{j}")
                eng = nc.gpsimd if j % 2 == 0 else nc.sync
                eng.dma_start(t[:], x32[:, j * MBLK:(j + 1) * MBLK])
                x32_t.append(t)

            def att_block(j, group, first):
                """att[j] (+)= sum_k hT'_k^T @ E_k[:, blk j]; first fold also
                adds the residual x."""
                blk = slice(j * MBLK, (j + 1) * MBLK)
                pa = psA.tile([P, MBLK], F32, tag="att")
                for k, (hk, ek) in enumerate(group):
                    nc.tensor.matmul(pa[:], hk[:], ek[:, blk],
                                     start=(k == 0), stop=(k == len(group) - 1))
                if first:
                    nc.vector.tensor_add(out=att_t[j][:], in0=pa[:],
                                         in1=x32_t[j][:])
                else:
                    nc.vector.tensor_add(out=att_t[j][:], in0=att_t[j][:],
                                         in1=pa[:])

            groups = [[] for _ in range(NWIN)]
            for w in range(NWIN):
                for i in range(GROUP):
                    s = w * GROUP + i
                    if i == 1 and w < NWIN - 1:
                        fg_block(w + 1, "f")
                    # hT = x_strip^T @ WhT + ones (x) bh  -> [n, c]; copied
                    # to SBUF right away so the psA slot frees quickly
                    ph = psA.tile([P, MBLK], F32, tag="att", name="ph")
                    nc.tensor.matmul(ph[:, 0:P], xbf_t[s // 4][:, (s % 4) * P:
                                                               (s % 4 + 1) * P],
                                     wht_s, start=True, stop=False)
                    nc.tensor.matmul(ph[:, 0:P], ones_s[:], bhr_s,
                                     start=False, stop=True)
                    ht0 = hpool.tile([P, P], BF16, tag="ht0", name="ht0")
                    nc.vector.tensor_copy(out=ht0[:], in_=ph[:, 0:P])
                    # S strip (K=16) -> exp -> E strip (+ accum partial sums)
                    e = epool.tile([P, N], BF16, tag="E")
                    # last strip: all chunks via ACT accum so the tail's
                    # attended chains aren't gated on the DVE rowsum tree
                    dvc = 0 if s == NSTRIP - 1 else len(CHUNKS) - 1
                    accs = small.tile([P, len(CHUNKS)], F32, tag="accs")
                    fl = f4_t[s // 4][:, (s % 4) * P:(s % 4 + 1) * P]
                    fsl = fl[0:CP, :]
                    for cix, (coff, cwd) in enumerate(CHUNKS):
                        if s == 0 and cix >= 1:
                            for gb in range(coff // MBLK,
                                            (coff + cwd) // MBLK):
                                fg_block(gb, "g")
                        sps = psS.tile([P, 1536], F32)
                        for half in range(cwd // MBLK):
                            off = half * MBLK
                            nc.tensor.matmul(
                                sps[:, off:off + MBLK],
                                fsl,
                                g4_t[cix][0:CP, off:off + MBLK],
                                start=True, stop=True)
                        eout = e[:, coff:coff + cwd]
                        # exp(S - 6): softmax is shift-invariant and the
                        # normalization uses the same shifted sums; keeps the
                        # fp16 rowsum tree far from overflow (exp(S) can
                        # exceed 6e4 at this problem's S scale)
                        if cix < dvc:
                            nc.scalar.activation(out=eout, in_=sps[:, 0:cwd],
                                                 func=EXP, bias=neg6_s[:])
                        else:
                            nc.scalar.activation(
                                out=eout, in_=sps[:, 0:cwd], func=EXP,
                                bias=neg6_s[:],
                                accum_out=accs[:, cix - dvc:cix - dvc + 1])
                    # D rowsum over chunks 0..2 via 2-byte DVE add-tree (2x
                    # packed mode: ~2.2us vs 3.3us plain reduce), chunk 3 from
                    # the ACT accumulator.  3072 -> 1536 -> 768 -> reduce.
                    d = small.tile([P, 1], F32, tag="d")
                    if dvc:
                        t1 = small.tile([P, 1536], F16, tag="t1")
                        nc.vector.tensor_add(out=t1[:], in0=e[:, 0:1536],
                                             in1=e[:, 1536:3072])
                        t2 = small.tile([P, 768], F16, tag="t2")
                        nc.vector.tensor_add(out=t2[:], in0=t1[:, 0:768],
                                             in1=t1[:, 768:1536])
                        dd = small.tile([P, 1], F32, tag="dd")
                        nc.vector.reduce_sum(out=dd[:], in_=t2[:], axis=AX)
                        nc.vector.tensor_add(out=d[:], in0=dd[:],
                                             in1=accs[:, 0:1])
                    else:
                        nc.vector.tensor_add(out=d[:], in0=accs[:, 0:1],
                                             in1=accs[:, 1:2])
                        nc.vector.tensor_add(out=d[:], in0=d[:],
                                             in1=accs[:, 2:3])
                    rd = small.tile([P, 1], F32, tag="rd")
                    nc.vector.reciprocal(out=rd[:], in_=d[:])
                    hts = hpool.tile([P, P], BF16, tag="hts")
                    nc.vector.tensor_scalar(out=hts[:], in0=ht0[:],
                                            scalar1=rd[:], scalar2=gam_s,
                                            op0=MUL, op1=MUL)
                    groups[w].append((hts, e))
                    # attended for the previous window's group (keeps PE busy
                    # while ACT chews the current window's exps)
                    if w >= 1:
                        for j in (2 * i, 2 * i + 1):
                            att_block(j, groups[w - 1], first=(w == 1))

            # tail: attended for the last group, then store. pa tiles come
            # from both psum pools (psS is idle now) for deeper overlap.
            for j in range(NMB):
                pool = psS if j % 2 == 0 else psA
                pa = pool.tile([P, 1536 if pool is psS else MBLK], F32,
                               tag="sps" if pool is psS else "att",
                               name=f"tailpa{j}")
                g = groups[-1]
                for k, (hk, ek) in enumerate(g):
                    nc.tensor.matmul(pa[:, 0:MBLK], hk[:],
                                     ek[:, j * MBLK:(j + 1) * MBLK],
                                     start=(k == 0), stop=(k == len(g) - 1))
                nc.vector.tensor_add(out=att_t[j][:], in0=att_t[j][:],
                                     in1=pa[:, 0:MBLK])
                nc.sync.dma_start(out[:, j * MBLK:(j + 1) * MBLK], att_t[j][:])

    nc.compile()
    return nc


def _get_nc():
    global _NC
    if _NC is None:
        _NC = _build()
    return _NC


def _prep_weights(Wf, bf, Wg, bg, Wh, bh, gamma):
    bf16 = ml_dtypes.bfloat16
    wft4 = np.zeros((P, P), np.float32)
    wgt4 = np.zeros((P, P), np.float32)
    bf4 = np.zeros((P, 1), np.float32)
    bg4 = np.zeros((P, 1), np.float32)
    for i in range(4):
        wft4[:, 32 * i:32 * i + CP] = Wf.T
        wgt4[:, 32 * i:32 * i + CP] = Wg.T
        bf4[32 * i:32 * i + CP, 0] = bf
        bg4[32 * i:32 * i + CP, 0] = bg
    wpack = np.zeros((P, 4 * P), np.float32)
    wpack[:, 0:P] = wft4
    wpack[:, P:2 * P] = wgt4
    wpack[:, 2 * P:3 * P] = Wh.T
    wpack[0, 3 * P:4 * P] = bh
    fpack = np.zeros((P, 3), np.float32)
    fpack[:, 0:1] = bf4
    fpack[:, 1:2] = bg4
    fpack[:, 2] = np.float32(np.asarray(gamma).reshape(()))
    return {"wpack": wpack.astype(bf16), "fpack": fpack}


def make_in_maps(x, Wf, bf, Wg, bg, Wh, bh, gamma):
    bf16 = ml_dtypes.bfloat16
    wmap = _prep_weights(np.asarray(Wf), np.asarray(bf), np.asarray(Wg),
                         np.asarray(bg), np.asarray(Wh), np.asarray(bh),
                         np.asarray(gamma))
    xf = np.ascontiguousarray(np.asarray(x, np.float32).reshape(B, C, N))
    in_maps = []
    for b in range(B):
        m = dict(wmap)
        m["x32"] = xf[b]
        m["xbf"] = xf[b].astype(bf16)
        in_maps.append(m)
    return in_maps


def kernel(x, Wf, bf, Wg, bg, Wh, bh, gamma):
    nc = _get_nc()
    in_maps = make_in_maps(x, Wf, bf, Wg, bg, Wh, bh, gamma)
    res = run_bass_kernel_spmd(nc, in_maps, core_ids=list(range(B)))
    out = np.stack([res.results[b]["out"] for b in range(B)], axis=0)
    return out.reshape(B, C, W, H).astype(np.float32)


# revision 30
# speedup vs baseline: 8316.6065x; 1.0044x over previous
"""NonLocal2D block (SAGAN-style non-local attention) on 8 Trainium2 cores.

Data-parallel over batch: core b computes batch element b entirely on-chip.

Math (per batch, N = 64*64 = 4096):
  f = Wf@x+bf [16,N], g = Wg@x+bg [16,N], h = Wh@x+bh [128,N]
  S = f^T g [N,N]; A = softmax_rows(S); att = h @ A; out = x + gamma*att

Decomposition (per core), using att[c,m] = sum_n hT'[n,c] * E[n,m] with
E = exp(S) and hT'[n,c] = h[c,n] * gamma/D[n], D[n] = sum_m E[n,m]:

  32 row-strips of 128 n's, in windows of 4. Per strip:
    S_strip = f_strip^T @ g          PE, K=16 bf16 matmuls -> PSUM
    E_strip = exp(S_strip)           ACT (the roofline: 16.7M exps/core),
                                     PSUM->SBUF bf16; last 2 chunks also
                                     emit accum_out partial row-sums
    D rowsum                         DVE reduce (first 2048 cols) + ACT accums
    hT = x_strip^T @ WhT + 1 (x) bh  PE (K=128 + K=1 rank-1 bias)
    hT' = hT * (gamma/D)             DVE, PSUM->SBUF bf16
  Attended accumulates over 4-strip groups in PSUM (K-chained matmuls) and
  is folded into an SBUF accumulator by DVE; group g's attended matmuls run
  during window g+1 so ACT never starves. The residual x is fused into the
  first fold; tail stores per 512-column block.

All tensors that are produced/consumed blockwise are split into per-block
tiles because Tile tracks dependencies per tile, not per slice.
"""

import numpy as np
import ml_dtypes

import concourse.bass as bass
import concourse.bacc as bacc
import concourse.tile as tile
import concourse.mybir as mybir
from concourse.bass_utils import run_bass_kernel_spmd

B, C, W, H = 8, 128, 64, 64
N = W * H          # 4096
CP = 16            # f/g channels
P = 128
NSTRIP = N // P    # 32
GROUP = 4          # strips per window / attended K-chain
NWIN = NSTRIP // GROUP      # 8
MBLK = 512
NMB = N // MBLK    # 8
# exp call chunks (psS tiles are [128,1536] = 3 banks x 2 bufs):
CHUNKS = [(0, 1536), (1536, 1536), (3072, 1024)]
# rowsum: chunks 0..1 (3072 cols) via DVE add-tree, chunk 2 via ACT accum

F32 = mybir.dt.float32
BF16 = mybir.dt.bfloat16
F16 = mybir.dt.float16
EXP = mybir.ActivationFunctionType.Exp
AX = mybir.AxisListType.X
MUL = mybir.AluOpType.mult

_NC = None


def _build():
    nc = bacc.Bacc(None, target_bir_lowering=False)
    x32 = nc.dram_tensor("x32", [P, N], F32, kind="ExternalInput")
    xbf = nc.dram_tensor("xbf", [P, N], BF16, kind="ExternalInput")
    # wpack: [wf^T rep | wg^T rep | wh^T | row0: bh] packed on host
    wpack = nc.dram_tensor("wpack", [P, 4 * P], BF16, kind="ExternalInput")
    # fpack: [bf4 | bg4 | gamma (pre-broadcast)] packed on host
    fpack = nc.dram_tensor("fpack", [P, 3], F32, kind="ExternalInput")
    out = nc.dram_tensor("out", [P, N], F32, kind="ExternalOutput")

    with tile.TileContext(nc) as tc:
        with (
            tc.tile_pool(name="consts", bufs=1) as consts,
            tc.tile_pool(name="epool", bufs=2 * GROUP + 4) as epool,
            tc.tile_pool(name="hpool", bufs=2 * GROUP + 6) as hpool,
            tc.tile_pool(name="small", bufs=8) as small,
            tc.tile_pool(name="psS", bufs=2, space="PSUM") as psS,
            tc.tile_pool(name="psA", bufs=2, space="PSUM") as psA,
        ):
            # ---- interleave input DMAs across the two DGE paths so xbf
            # block 0 and the packed weights land first.
            wpack_s = consts.tile([P, 4 * P], BF16)
            fpack_s = consts.tile([P, 3], F32)
            xbf_t = [consts.tile([P, MBLK], BF16, tag=f"xbf{j}", name=f"xbf{j}")
                     for j in range(NMB)]
            nc.sync.dma_start(wpack_s[:, 0:2 * P], wpack[:, 0:2 * P])
            nc.gpsimd.dma_start(xbf_t[0][:], xbf[:, 0:MBLK])
            nc.sync.dma_start(fpack_s[:], fpack[:])
            nc.gpsimd.dma_start(wpack_s[:, 2 * P:4 * P], wpack[:, 2 * P:4 * P])
            nc.sync.dma_start(xbf_t[1][:], xbf[:, MBLK:2 * MBLK])
            for j in range(2, NMB):
                eng = nc.gpsimd if j % 2 == 0 else nc.sync
                eng.dma_start(xbf_t[j][:], xbf[:, j * MBLK:(j + 1) * MBLK])

            wft4_s = wpack_s[:, 0:P]
            wgt4_s = wpack_s[:, P:2 * P]
            wht_s = wpack_s[:, 2 * P:3 * P]
            bhr_s = wpack_s[0:1, 3 * P:4 * P]
            bf4_s = fpack_s[:, 0:1]
            bg4_s = fpack_s[:, 1:2]
            gam_s = fpack_s[:, 2:3]
            ones_s = consts.tile([1, P], BF16)
            nc.vector.memset(ones_s[:], 1.0)
            neg6_s = consts.tile([P, 1], F32)
            nc.vector.memset(neg6_s[:], -6.0)
            # dummy exp with no input deps: pulls the ACT table load to t=0
            # instead of just before the first real activation
            warm = small.tile([P, 1], F32, tag="warm")
            nc.scalar.activation(out=warm[:], in_=neg6_s[:], func=EXP)

            f4_t = [consts.tile([P, MBLK], BF16, tag=f"f4{j}", name=f"f4{j}")
                    for j in range(NMB)]
            g4_t = [consts.tile([P, wd], BF16, tag=f"g4{c}", name=f"g4{c}")
                    for c, (off, wd) in enumerate(CHUNKS)]
            att_t = [consts.tile([P, MBLK], F32, tag=f"att{j}", name=f"att{j}")
                     for j in range(NMB)]

            # ---- f/g 1x1 convs; bias added on the PSUM->SBUF copy.
            # Order matters: strip 0 needs f block 0 and the g chunks in
            # order, so emit those first; remaining f blocks trail.
            IDENT = mybir.ActivationFunctionType.Identity

            def fg_block(j, which, via_act=False):
                ps = psA.tile([P, MBLK], F32, tag="att")
                if which == "f":
                    dst, b = f4_t[j][:], bf4_s
                    nc.tensor.matmul(ps[:], wft4_s, xbf_t[j][:],
                                     start=True, stop=True)
                else:
                    c = next(i for i, (off, wd) in enumerate(CHUNKS)
                             if off <= j * MBLK < off + wd)
                    o = j * MBLK - CHUNKS[c][0]
                    dst = g4_t[c][:, o:o + MBLK]
                    b = bg4_s
                    nc.tensor.matmul(ps[:], wgt4_s, xbf_t[j][:],
                                     start=True, stop=True)
                if via_act:
                    # ACT is idle during startup; Identity shares Exp's table
                    nc.scalar.activation(out=dst, in_=ps[:], func=IDENT,
                                         bias=b, scale=1.0)
                else:
                    nc.vector.tensor_scalar_add(out=dst, in0=ps[:], scalar1=b)

            # Only what strip 0 chunk 0 needs; the rest is emitted
            # just-in-time inside the strip loop (PE executes in order, so
            # early emission would delay strip 0's S matmuls).
            fg_block(0, "f", via_act=True)
            fg_block(0, "g", via_act=False)
            fg_block(1, "g", via_act=True)
            fg_block(2, "g", via_act=False)

            # x32 only needed for the first folds; per-block tiles
            x32_t = []
            for j in range(NMB):
                t = consts.tile([P, MBLK], F32, tag=f"x32{j}", name=f"x32{j}")
                eng = nc.gpsimd if j % 2 == 0 else nc.sync
                eng.dma_start(t[:], x32[:, j * MBLK:(j + 1) * MBLK])
                x32_t.append(t)

            def att_block(j, group, first):
                """att[j] (+)= sum_k hT'_k^T @ E_k[:, blk j]; first fold also
                adds the residual x."""
                blk = slice(j * MBLK, (j + 1) * MBLK)
                pa = psA.tile([P, MBLK], F32, tag="att")
                for k, (hk, ek) in enumerate(group):
                    nc.tensor.matmul(pa[:], hk[:], ek[:, blk],
                                     start=(k == 0), stop=(k == len(group) - 1))
                if first:
                    nc.vector.tensor_add(out=att_t[j][:], in0=pa[:],
                                         in1=x32_t[j][:])
                else:
                    nc.vector.tensor_add(out=att_t[j][:], in0=att_t[j][:],
                                         in1=pa[:])

            groups = [[] for _ in range(NWIN)]
            for w in range(NWIN):
                for i in range(GROUP):
                    s = w * GROUP + i
                    if i == 1 and w < NWIN - 1:
                        fg_block(w + 1, "f")
                    # S strip (K=16) -> exp -> E strip (+ accum partial sums)
                    e = epool.tile([P, N], BF16, tag="E")
                    # last strip: all chunks via ACT accum so the tail's
                    # attended chains aren't gated on the DVE rowsum tree
                    dvc = 0 if s == NSTRIP - 1 else len(CHUNKS) - 1
                    accs = small.tile([P, len(CHUNKS)], F32, tag="accs")
                    fl = f4_t[s // 4][:, (s % 4) * P:(s % 4 + 1) * P]
                    fsl = fl[0:CP, :]
                    for cix, (coff, cwd) in enumerate(CHUNKS):
                        if s == 0 and cix >= 1:
                            for gb in range(coff // MBLK,
                                            (coff + cwd) // MBLK):
                                fg_block(gb, "g")
                        sps = psS.tile([P, 1536], F32)
                        for half in range(cwd // MBLK):
                            off = half * MBLK
                            nc.tensor.matmul(
                                sps[:, off:off + MBLK],
                                fsl,
                                g4_t[cix][0:CP, off:off + MBLK],
                                start=True, stop=True)
                        eout = e[:, coff:coff + cwd]
                        # exp(S - 6): softmax is shift-invariant and the
                        # normalization uses the same shifted sums; keeps the
                        # fp16 rowsum tree far from overflow (exp(S) can
                        # exceed 6e4 at this problem's S scale)
                        if cix < dvc:
                            nc.scalar.activation(out=eout, in_=sps[:, 0:cwd],
                                                 func=EXP, bias=neg6_s[:])
                        else:
                            nc.scalar.activation(
                                out=eout, in_=sps[:, 0:cwd], func=EXP,
                                bias=neg6_s[:],
                                accum_out=accs[:, cix - dvc:cix - dvc + 1])
                    # hT = x_strip^T @ WhT + ones (x) bh  -> [n, c]; after
                    # the S chunks so it never delays ACT's food; copied to
                    # SBUF right away so the psA slot frees quickly
                    ph = psA.tile([P, MBLK], F32, tag="att", name="ph")
                    nc.tensor.matmul(ph[:, 0:P], xbf_t[s // 4][:, (s % 4) * P:
                                                               (s % 4 + 1) * P],
                                     wht_s, start=True, stop=False)
                    nc.tensor.matmul(ph[:, 0:P], ones_s[:], bhr_s,
                                     start=False, stop=True)
                    ht0 = hpool.tile([P, P], BF16, tag="ht0", name="ht0")
                    nc.vector.tensor_copy(out=ht0[:], in_=ph[:, 0:P])
                    # D rowsum over chunks 0..2 via 2-byte DVE add-tree (2x
                    # packed mode: ~2.2us vs 3.3us plain reduce), chunk 3 from
                    # the ACT accumulator.  3072 -> 1536 -> 768 -> reduce.
                    d = small.tile([P, 1], F32, tag="d")
                    if dvc:
                        t1 = small.tile([P, 1536], F16, tag="t1")
                        nc.vector.tensor_add(out=t1[:], in0=e[:, 0:1536],
                                             in1=e[:, 1536:3072])
                        t2 = small.tile([P, 768], F16, tag="t2")
                        nc.vector.tensor_add(out=t2[:], in0=t1[:, 0:768],
                                             in1=t1[:, 768:1536])
                        t3 = small.tile([P, 384], F16, tag="t3")
                        nc.vector.tensor_add(out=t3[:], in0=t2[:, 0:384],
                                             in1=t2[:, 384:768])
                        dd = small.tile([P, 1], F32, tag="dd")
                        nc.vector.reduce_sum(out=dd[:], in_=t3[:], axis=AX)
                        nc.vector.tensor_add(out=d[:], in0=dd[:],
                                             in1=accs[:, 0:1])
                    else:
                        nc.vector.tensor_add(out=d[:], in0=accs[:, 0:1],
                                             in1=accs[:, 1:2])
                        nc.vector.tensor_add(out=d[:], in0=d[:],
                                             in1=accs[:, 2:3])
                    rd = small.tile([P, 1], F32, tag="rd")
                    nc.vector.reciprocal(out=rd[:], in_=d[:])
                    hts = hpool.tile([P, P], BF16, tag="hts")
                    nc.vector.tensor_scalar(out=hts[:], in0=ht0[:],
                                            scalar1=rd[:], scalar2=gam_s,
                                            op0=MUL, op1=MUL)
                    groups[w].append((hts, e))
                    # attended for the previous window's group (keeps PE busy
                    # while ACT chews the current window's exps)
                    if w >= 1:
                        for j in (2 * i, 2 * i + 1):
                            att_block(j, groups[w - 1], first=(w == 1))

            # tail: attended for the last group, then store. pa tiles come
            # from both psum pools (psS is idle now) for deeper overlap.
            for j in range(NMB):
                pool = psS if j % 2 == 0 else psA
                pa = pool.tile([P, 1536 if pool is psS else MBLK], F32,
                               tag="sps" if pool is psS else "att",
                               name=f"tailpa{j}")
                g = groups[-1]
                for k, (hk, ek) in enumerate(g):
                    nc.tensor.matmul(pa[:, 0:MBLK], hk[:],
                                     ek[:, j * MBLK:(j + 1) * MBLK],
                                     start=(k == 0), stop=(k == len(g) - 1))
                nc.vector.tensor_add(out=att_t[j][:], in0=att_t[j][:],
                                     in1=pa[:, 0:MBLK])
                nc.sync.dma_start(out[:, j * MBLK:(j + 1) * MBLK], att_t[j][:])

    nc.compile()
    return nc


def _get_nc():
    global _NC
    if _NC is None:
        _NC = _build()
    return _NC


def _prep_weights(Wf, bf, Wg, bg, Wh, bh, gamma):
    bf16 = ml_dtypes.bfloat16
    wft4 = np.zeros((P, P), np.float32)
    wgt4 = np.zeros((P, P), np.float32)
    bf4 = np.zeros((P, 1), np.float32)
    bg4 = np.zeros((P, 1), np.float32)
    for i in range(4):
        wft4[:, 32 * i:32 * i + CP] = Wf.T
        wgt4[:, 32 * i:32 * i + CP] = Wg.T
        bf4[32 * i:32 * i + CP, 0] = bf
        bg4[32 * i:32 * i + CP, 0] = bg
    wpack = np.zeros((P, 4 * P), np.float32)
    wpack[:, 0:P] = wft4
    wpack[:, P:2 * P] = wgt4
    wpack[:, 2 * P:3 * P] = Wh.T
    wpack[0, 3 * P:4 * P] = bh
    fpack = np.zeros((P, 3), np.float32)
    fpack[:, 0:1] = bf4
    fpack[:, 1:2] = bg4
    fpack[:, 2] = np.float32(np.asarray(gamma).reshape(()))
    return {"wpack": wpack.astype(bf16), "fpack": fpack}


def make_in_maps(x, Wf, bf, Wg, bg, Wh, bh, gamma):
    bf16 = ml_dtypes.bfloat16
    wmap = _prep_weights(np.asarray(Wf), np.asarray(bf), np.asarray(Wg),
                         np.asarray(bg), np.asarray(Wh), np.asarray(bh),
                         np.asarray(gamma))
    xf = np.ascontiguousarray(np.asarray(x, np.float32).reshape(B, C, N))
    in_maps = []
    for b in range(B):
        m = dict(wmap)
        m["x32"] = xf[b]
        m["xbf"] = xf[b].astype(bf16)
        in_maps.append(m)
    return in_maps


def kernel(x, Wf, bf, Wg, bg, Wh, bh, gamma):
    nc = _get_nc()
    in_maps = make_in_maps(x, Wf, bf, Wg, bg, Wh, bh, gamma)
    res = run_bass_kernel_spmd(nc, in_maps, core_ids=list(range(B)))
    out = np.stack([res.results[b]["out"] for b in range(B)], axis=0)
    return out.reshape(B, C, W, H).astype(np.float32)
